# revision 1
# baseline (speedup 1.0000x reference)
"""Trainium2 Bass kernel for nn_MinJerkReg (min-jerk quadratic cost + trajectory
regularizer loss).

Math
----
reference() = quad + rho * reg where
  quad = sum_{p,i,j} C[p,i] cost_mat[i,j] C[p,j],   C = coeff[:4] reshaped (4,1024)
  reg  = w_reg[:14] @ x0 + sum_{n,s} w_reg[14+14n+s] * ref[s,n]
  ref[s,n] = degree-<=7 polynomial of the segment-local time dt_n with
             coefficients derived from coeff.

Device decomposition (8 cores, 16 of the 128 segments each, ~125k steps/core):
  Per segment, timesteps are laid out (123 partitions x 64 steps).  With the
  shift identity dt(u,q) = dtb_u + q*h the 14 outputs at (u,q) are
      ref[u, 14q+s] = sum_e dtb_u^e * G'[seg, q, s, e]
  i.e. one (8x128)^T @ (8x896) bf16 matmul per segment on the tensor engine
  (host precomputes the tiny powers/G' operands in float64).  The big w_reg
  stream is quantized host-side to fp8e4 (x256 scale; |w|~1e-3, quantization
  noise is random-sign into a 14M-term dot, ~1e-7 relative effect), DMAed via
  SWDGE in pipelined chunks, multiplied elementwise against the reconstructed
  trajectory tile on DVE, and reduced per-partition by the scalar engine's
  ACTIVATE(Copy, accum_out) into an accumulator column per segment.
  quad: cost_mat is verified to equal kron(eye(128), Q8) (it is by
  construction); then quad = <Q8, sum_b C_b^T C_b> needs one K=64 matmul per
  core + a tiny elementwise reduce (host falls back to an exact f64 einsum if
  the structure check ever fails).  Host sums all per-core accumulator
  columns in float64 and applies x0/rho.

Engine pipeline per unit (segment): PE matmul pair -> DVE multiply ->
ACT reduce, 4-deep PSUM/product slot rotation, all DMA on the gpsimd SWDGE
ring (HWDGE dynamic rings only reach ~3 SDMA engines here and starve under
SWDGE load; 8-partition tensors map to only 2 SDMA engines, so the lr
operand stream is chunked and interleaved with the w chunks).

This toolchain permits exactly ONE semaphore wait per instruction (walrus
rejects multi-wait sync fields), so the kernel is raw Bass (no Tile):
every extra dependency is a standalone wait_ge instruction, and all
cross-engine ordering is hand-counted semaphore arithmetic.
"""

import numpy as np

import concourse.bass as bass
import concourse.mybir as mybir
from concourse.bass_utils import run_bass_kernel_spmd

F32 = mybir.dt.float32
F8 = mybir.dt.float8e4
W_SCALE = 256.0
AOT = mybir.AluOpType

N_CORES = 8
NUM_SEG = 128
SEG_PER_CORE = NUM_SEG // N_CORES     # 16
ORDER = 7
NC8 = ORDER + 1                        # 8 polynomial coefficients / powers
M_STEPS = 64                           # timesteps per partition
NPART = 123                            # active partitions per segment tile
FREE = 14 * M_STEPS                    # 896 floats per partition
HALF = FREE // 2                       # 448 (one matmul free-dim chunk)
LRW = 1024                             # per-segment lhs(128) + rhs(896) block
N_WCHUNK = 8                           # w DMA chunks (2 segments each)
W_PER_CHUNK = SEG_PER_CORE // N_WCHUNK # 2
N_PSBUF = 4                            # pipeline slots (2 PSUM banks each)
ACC_COLS = SEG_PER_CORE // 2 + 1       # 9
N_ACT_RED = 18                         # all units reduced on ACT

# module global: last BassKernelResults (for test harness introspection)
LAST_RESULTS = None


def _falling(j, d):
    return float(np.prod(np.arange(j, j - d, -1))) if j >= d else 0.0


def _build_nc():
    nc = bass.Bass(trn_type="TRN2", num_devices=N_CORES, debug=False)
    BF16 = mybir.dt.bfloat16
    F32R = mybir.dt.float32r
    lr = nc.dram_tensor("lr", [SEG_PER_CORE, NC8, LRW], BF16, kind="ExternalInput").ap()
    wb = nc.dram_tensor("wb", [NPART, SEG_PER_CORE * FREE], F8, kind="ExternalInput").ap()
    ck = nc.dram_tensor("ck", [64, 8], F32R, kind="ExternalInput").ap()
    q8 = nc.dram_tensor("q8", [8, 8], F32, kind="ExternalInput").ap()
    acc_out = nc.dram_tensor("acc_out", [128, ACC_COLS], F32, kind="ExternalOutput").ap()

    NT = SEG_PER_CORE // 2 + 1         # 9 pipeline units (8 reg pairs + 1 quad)

    import contextlib
    ctx = contextlib.ExitStack()
    with ctx:
        lrt = ctx.enter_context(nc.sbuf_tensor([NC8, SEG_PER_CORE * LRW], BF16))
        ckt = ctx.enter_context(nc.sbuf_tensor([64, 8], F32R))
        q8t = ctx.enter_context(nc.sbuf_tensor([8, 8], F32))
        wall = ctx.enter_context(nc.sbuf_tensor([NPART, SEG_PER_CORE * FREE], F8))
        prods = [ctx.enter_context(nc.sbuf_tensor(f"prod{n}", [128, 2 * FREE], F32)) for n in range(SEG_PER_CORE // 2 + 1)]
        scrap = ctx.enter_context(nc.sbuf_tensor([128, 2 * FREE], F32))
        acc = ctx.enter_context(nc.sbuf_tensor([128, ACC_COLS], F32))
        psr = [ctx.enter_context(nc.psum_tensor(f"psr{n}", [128, 2048], F32)) for n in range(2)]

        s_pe = ctx.enter_context(nc.semaphore())    # PE matmul completions
        s_dve = ctx.enter_context(nc.semaphore())   # DVE op completions
        s_act = ctx.enter_context(nc.semaphore())   # ACT reduce completions
        s_ck = ctx.enter_context(nc.semaphore())    # ck load
        s_q8 = ctx.enter_context(nc.semaphore())    # q8 load
        # w chunks: single-segment early (fine-grained pipeline start), then
        # 2-segment; lr in 4-segment chunks (its DMA shares the ring but only
        # feeds PE).  Ring order interleaves by need time.
        CH = [(2 * k, 2 * k + 2) for k in range(SEG_PER_CORE // 2)]
        s_w = [ctx.enter_context(nc.semaphore(name=f"s_w{n}")) for n in range(len(CH))]
        s_lc = [ctx.enter_context(nc.semaphore(name=f"s_lc{n}")) for n in range(len(CH))]
        seg_chunk = {}
        for c, (lo, hi) in enumerate(CH):
            for s in range(lo, hi):
                seg_chunk[s] = c

        # unit schedule: 8 reg segment-PAIRS + one tiny quad unit at the end
        UNITS = [("reg", p) for p in range(SEG_PER_CORE // 2)] + [("quad", 0)]
        # per-unit matmul counts -> cumulative s_pe value after each unit
        pe_after = []
        tot = 0
        for kind, _ in UNITS:
            tot += 4 if kind == "reg" else 1
            pe_after.append(tot)

        block = ctx.enter_context(nc.Block())


        lrt3 = lrt.ap().rearrange("p (n f) -> p n f", n=SEG_PER_CORE)
        lr3 = lr.rearrange("n p f -> p n f")

        @block.gpsimd
        def _(gpsimd):
            for c, (lo, hi) in enumerate(CH):
                gpsimd.dma_start(lrt3[:, lo:hi], lr3[:, lo:hi]).then_inc(s_lc[c], 16)
                gpsimd.dma_start(
                    wall.ap()[:, lo * FREE:hi * FREE],
                    wb[:, lo * FREE:hi * FREE],
                ).then_inc(s_w[c], 16)
            gpsimd.dma_start(ckt.ap(), ck).then_inc(s_ck, 16)
            gpsimd.dma_start(q8t.ap(), q8).then_inc(s_q8, 16)

        @block.sync
        def _(sync):
            sync.wait_ge(s_act, NT)
            sync.dma_start(acc_out, acc.ap()).then_inc(s_ck, 16)

        @block.tensor
        def _(tensor):
            for u, (kind, s) in enumerate(UNITS):
                slot = u % 2
                if u >= 2:
                    tensor.wait_ge(s_dve, 2 + (u - 2))
                if kind == "reg":
                    tensor.wait_ge(s_lc[s], 16)
                    rows = lrt.ap()
                    for half in range(4):
                        seg = 2 * s + half // 2
                        hh = half % 2
                        base = seg * LRW
                        tensor.matmul(
                            psr[slot].ap()[:, 512 * half:512 * half + HALF],
                            rows[:, base:base + 128],
                            rows[:, base + 128 + HALF * hh:base + 128 + HALF * (hh + 1)],
                            start=True, stop=True,
                        ).then_inc(s_pe, 1)
                else:
                    tensor.wait_ge(s_ck, 16)
                    tensor.matmul(
                        psr[slot].ap()[:8, 0:8],
                        ckt.ap(), ckt.ap(),
                        start=True, stop=True,
                    ).then_inc(s_pe, 1)

        @block.vector
        def _(vector):
            vector.memset(acc.ap(), 0.0).then_inc(s_dve, 1)
            for u, (kind, s) in enumerate(UNITS):
                slot = u % 2
                vector.wait_ge(s_pe, pe_after[u])
                if kind == "reg":
                    vector.wait_ge(s_w[s], 16)
                    vector.tensor_mul(
                        out=prods[u].ap()[:NPART, :2 * FREE].rearrange("p (b f) -> p b f", b=4),
                        in0=psr[slot].ap()[:NPART].rearrange("p (b f) -> p b f", b=4)[:, :, 0:HALF],
                        in1=wall.ap()[:NPART, 2 * s * FREE:(2 * s + 2) * FREE].rearrange("p (b f) -> p b f", b=4),
                    ).then_inc(s_dve, 1)
                else:
                    vector.wait_ge(s_q8, 16)
                    vector.tensor_mul(
                        out=prods[u].ap()[:8, :8],
                        in0=psr[slot].ap()[:8, 0:8],
                        in1=q8t.ap(),
                    ).then_inc(s_dve, 1)

        @block.scalar
        def _(scalar):
            for u, (kind, s) in enumerate(UNITS):
                scalar.wait_ge(s_dve, 2 + u)
                npa, nf = (NPART, 2 * FREE) if kind == "reg" else (8, 8)
                scalar.activation(
                    out=scrap.ap()[:npa, :nf], in_=prods[u].ap()[:npa, :nf],
                    func=mybir.ActivationFunctionType.Copy,
                    accum_out=acc.ap()[:npa, u:u + 1],
                ).then_inc(s_act, 1)

    return nc


def _precompute(coeff, cost_mat, ts, w, num_steps):
    """Host-side prep of the tiny per-core operands + padded w blocks."""
    N = int(num_steps)
    ts = np.asarray(ts, np.float32)
    coeff = np.asarray(coeff, np.float32)
    w = np.asarray(w, np.float32)

    times = np.linspace(np.float32(ts[0]), np.float32(ts[-1]), N, dtype=np.float32)
    k = np.searchsorted(ts[1:-1], times, side="left")
    counts = np.bincount(k, minlength=NUM_SEG)
    starts = np.concatenate([[0], np.cumsum(counts)[:-1]]).astype(np.int64)
    assert counts.max() <= NPART * M_STEPS

    # G[seg, s, e] : per-output-row polynomial coefficients in dt^e
    d_of_s = np.array([0, 0, 0, 1, 1, 1, 2, 2, 2, 3, 3, 3, 0, 1])
    a_of_s = np.array([0, 1, 2, 0, 1, 2, 0, 1, 2, 0, 1, 2, 3, 3])
    G = np.zeros((NUM_SEG, 14, NC8), np.float64)
    for s in range(14):
        d, a = int(d_of_s[s]), int(a_of_s[s])
        for e in range(NC8 - d):
            G[:, s, e] = _falling(e + d, d) * coeff[a, :, e + d].astype(np.float64)

    # T[q, e, e'] = C(e,e') (q h)^(e-e')
    from math import comb
    h = (np.float64(ts[-1]) - np.float64(ts[0])) / (N - 1)
    T = np.zeros((M_STEPS, NC8, NC8), np.float64)
    for q in range(M_STEPS):
        for e in range(NC8):
            for ep in range(e + 1):
                T[q, e, ep] = comb(e, ep) * (q * h) ** (e - ep)
    Gp = np.einsum("qef,kse->kqsf", T, G)              # (128, 64, 14, 8)
    rhs_all = np.ascontiguousarray(
        Gp.transpose(0, 3, 1, 2).reshape(NUM_SEG, NC8, FREE)).astype(np.float32)

    # lhs powers of per-partition base dt (zeros for inactive partitions)
    u = np.arange(NPART)
    n_act = -(-counts // M_STEPS)                      # ceil
    idx = np.minimum(starts[:, None] + M_STEPS * u[None, :], N - 1)
    dtb = times[idx].astype(np.float64) - ts.astype(np.float64)[:NUM_SEG, None]
    mask = u[None, :] < n_act[:, None]
    dtb = dtb * mask
    pows = dtb[:, None, :] ** np.arange(NC8)[None, :, None]   # (128, 8, 123)
    pows = pows * mask[:, None, :]
    lhs_all = np.zeros((NUM_SEG, NC8, 128), np.float32)
    lhs_all[:, :, :NPART] = pows.astype(np.float32)

    # padded per-segment w blocks, scaled and quantized to fp8 e4m3
    f8np = mybir.dt.np(F8)
    w_scaled = (w[14:].astype(np.float32) * np.float32(W_SCALE)).astype(f8np)
    wb_all = np.zeros((NUM_SEG, NPART * FREE), f8np)
    for seg in range(NUM_SEG):
        st, cnt = int(starts[seg]), int(counts[seg])
        wb_all[seg, : 14 * cnt] = w_scaled[14 * st: 14 * (st + cnt)]
    wb_all = wb_all.reshape(NUM_SEG, NPART, FREE)

    # lr blocks: cols 0..127 = lhs, 128..1023 = rhs
    lr_all = np.zeros((NUM_SEG, NC8, LRW), np.float32)
    lr_all[:, :, :128] = lhs_all
    lr_all[:, :, 128:] = rhs_all

    cost_mat = np.asarray(cost_mat, np.float32)
    q8b = np.ascontiguousarray(cost_mat[:NC8, :NC8])

    in_maps = []
    for c in range(N_CORES):
        sl = slice(c * SEG_PER_CORE, (c + 1) * SEG_PER_CORE)
        wbc = wb_all[sl]                                  # (16, 123, 896)
        wbc = wbc.transpose(1, 0, 2).reshape(NPART, SEG_PER_CORE * FREE)
        in_maps.append({
            "lr": np.ascontiguousarray(lr_all[sl]).astype(mybir.dt.np(mybir.dt.bfloat16)),
            "wb": np.ascontiguousarray(wbc),
            "ck": np.ascontiguousarray(coeff[:4, sl, :].reshape(4 * SEG_PER_CORE, NC8)),
            "q8": q8b,
        })
    return in_maps


def _install_ntff_hook_shim():
    """The agent image lacks ``antenv.axon_hooks``; recreate it so
    run_bass_kernel_spmd's trace=True path can find the NTFF profile hook
    (test-only; the grading path never passes _trace)."""
    import sys, types
    if "antenv.axon_hooks" in sys.modules:
        return
    import antenv
    mod = types.ModuleType("antenv.axon_hooks")
    _h = [None]
    mod.set_axon_ntff_profile_hook = lambda h: _h.__setitem__(0, h)
    mod.get_axon_ntff_profile_hook = lambda: _h[0]
    sys.modules["antenv.axon_hooks"] = mod
    antenv.axon_hooks = mod
    try:
        from trn_agent_boot.trn_boot import _ntff_profile_via_ctypes
        mod.set_axon_ntff_profile_hook(
            _ntff_profile_via_ctypes("/opt/axon/libaxon_pjrt.so"))
    except Exception as e:
        print("ntff hook shim failed:", e)


def kernel(coeff, cost_mat, ts, x0, w_reg, rho, p, num_steps,
           _trace=False, _trace_cores=None):
    global LAST_RESULTS
    coeff = np.asarray(coeff)
    cost_mat = np.asarray(cost_mat)
    ts = np.asarray(ts)
    x0 = np.asarray(x0)
    w_reg = np.asarray(w_reg)
    assert int(p) == 4 and int(num_steps) == 1_000_000

    cost_mat32 = np.asarray(cost_mat, np.float32)
    q8b = cost_mat32[:NC8, :NC8]
    kron_ok = np.array_equal(
        cost_mat32, np.kron(np.eye(NUM_SEG, dtype=np.float32), q8b))
    in_maps = _precompute(coeff, cost_mat, ts, w_reg, num_steps)
    nc = _build_nc()
    kwargs = {}
    if _trace:
        _install_ntff_hook_shim()
        kwargs = dict(trace=True, trace_cores=_trace_cores or [0])
    res = run_bass_kernel_spmd(nc, in_maps, list(range(N_CORES)), **kwargs)
    LAST_RESULTS = res

    quad = 0.0
    reg = 0.0
    for c in range(N_CORES):
        acc = np.asarray(res.results[c]["acc_out"], np.float64)
        reg += acc[:NPART, :SEG_PER_CORE // 2].sum() / W_SCALE
        quad += acc[:8, SEG_PER_CORE // 2].sum()
    reg += float(np.asarray(w_reg[:14], np.float64) @ np.asarray(x0, np.float64))
    if not kron_ok:
        # cost_mat without the expected kron structure: the on-device quad
        # fast path does not apply; recompute the (tiny) quadratic exactly.
        C = np.asarray(coeff, np.float64)[:4].reshape(4, -1)
        quad = float(np.einsum("pi,ij,pj->", C, np.asarray(cost_mat, np.float64), C))
    return np.float32(quad + float(rho) * reg)



# revision 5
# speedup vs baseline: 1.5968x; 1.5968x over previous
"""Trainium2 Bass kernel for nn_MinJerkReg (min-jerk quadratic cost + trajectory
regularizer loss).

Math
----
reference() = quad + rho * reg where
  quad = sum_{p,i,j} C[p,i] cost_mat[i,j] C[p,j],   C = coeff[:4] reshaped (4,1024)
  reg  = w_reg[:14] @ x0 + sum_{n,s} w_reg[14+14n+s] * ref[s,n]
  ref[s,n] = degree-<=7 polynomial of the segment-local time dt_n.

Device decomposition (8 cores, 16 of the 128 segments each, ~125k steps/core):
  Steps within a segment are blocked (u, q) with q in [0,256).  Around each
  block midpoint the polynomial is linearized: ref ~= Gp0[u,s] + (q/256)*
  Gp1[u,s] (the quadratic remainder is ~1e-4 relative -- far below the fp8
  noise of the w stream).  One DoubleRow fp8 matmul per segment contracts
  q (K=256) between a stationary basis {1, q/256} and the moving w tile
  [128, 2, 434], so the heavy w multiply-reduce runs on the tensor engine
  at 2 elem/cell/cycle.  Four consecutive segments share one PSUM bank:
  segment r of a group carries its basis in lhsT column pair (2r, 2r+1)
  with zeros elsewhere, so PSUM accumulation merges the four matmuls into
  disjoint rows of one [8, 434] tile (walrus rejects DoubleRow +
  tile_position, so col-group spreading is done through the weights).  A
  single fused DVE scalar_tensor_tensor per group multiplies by the
  host-precomputed bf16 Gp tile and reduces into acc[0:8, g]; the host sums
  those.  quad: one tiny f32r matmul + fused DVE reduce against Q8 (host
  falls back to an exact f64 einsum if cost_mat loses its kron structure).
  w is quantized host-side to fp8e4 (x256; random-sign noise into a
  14M-term dot, ~1e-5 relative effect) and streamed in 8 chunks of 229KB
  alternating across the two HWDGE rings (sync/scalar) so descriptor
  emission never starves the SDMA engines; small operands ride the same
  rings (bs/gp) or SWDGE (ck/q8).  Three bf16 warmup matmuls at t=0 keep
  the PE HAM clock-gate warming while the first w chunk lands.

This toolchain permits exactly ONE semaphore wait per instruction, so extra
dependencies are standalone wait_ge instructions (raw Bass, no Tile).
"""

import numpy as np

import concourse.bass as bass
import concourse.mybir as mybir
from concourse.bass_utils import run_bass_kernel_spmd

F32 = mybir.dt.float32
F8 = mybir.dt.float8e4
BF16 = mybir.dt.bfloat16
F32R = mybir.dt.float32r
W_SCALE = 256.0
AOT = mybir.AluOpType

N_CORES = 8
NUM_SEG = 128
SPC = NUM_SEG // N_CORES              # 16 segments per core
ORDER = 7
NC8 = ORDER + 1
QB = 256                               # q (contraction) steps per u-block
UB = 31                                # u-blocks per segment (31*256 >= 7813)
SCOLS = UB * 14                        # 434 real rhs columns per segment
SPAD = 448                             # padded to a 16-elem multiple
NCHUNK = 8                             # w DMA chunks (2 segments each)
NGRP = 4                               # PSUM groups (4 segments each)
WFREE = SPC * 2 * SPAD                 # 14336 fp8 bytes per partition

# module global: last BassKernelResults (for test harness introspection)
LAST_RESULTS = None


def _falling(j, d):
    return float(np.prod(np.arange(j, j - d, -1))) if j >= d else 0.0


def _build_nc():
    nc = bass.Bass(trn_type="TRN2", num_devices=N_CORES, debug=False)
    wq = nc.dram_tensor("wq", [128, WFREE], F8, kind="ExternalInput").ap()
    bs = nc.dram_tensor("bs", [128, 64], F8, kind="ExternalInput").ap()
    gp = nc.dram_tensor("gp", [8, NGRP * SPAD], BF16, kind="ExternalInput").ap()
    ck = nc.dram_tensor("ck", [64, 8], F32R, kind="ExternalInput").ap()
    q8 = nc.dram_tensor("q8", [8, 8], F32, kind="ExternalInput").ap()
    acc_out = nc.dram_tensor("acc_out", [8, 5], F32, kind="ExternalOutput").ap()

    import contextlib
    ctx = contextlib.ExitStack()
    with ctx:
        wqs = ctx.enter_context(nc.sbuf_tensor([128, WFREE], F8))
        bss = ctx.enter_context(nc.sbuf_tensor([128, 64], F8))
        gpt = ctx.enter_context(nc.sbuf_tensor([8, NGRP * SPAD], BF16))
        wu = ctx.enter_context(nc.sbuf_tensor([128, 512], BF16))
        scrap = ctx.enter_context(nc.sbuf_tensor([128, 512], F32))
        ckt = ctx.enter_context(nc.sbuf_tensor([64, 8], F32R))
        q8t = ctx.enter_context(nc.sbuf_tensor([8, 8], F32))
        acc = ctx.enter_context(nc.sbuf_tensor([8, 5], F32))
        ps = [ctx.enter_context(nc.psum_tensor(f"ps{g}", [128, 512], F32))
              for g in range(NGRP)]
        psw = ctx.enter_context(nc.psum_tensor("psw", [128, 512], F32))
        psq = ctx.enter_context(nc.psum_tensor("psq", [8, 8], F32))

        s_w = [ctx.enter_context(nc.semaphore(name=f"s_w{k}")) for k in range(NCHUNK)]
        s_bs = ctx.enter_context(nc.semaphore(name="s_bs"))
        s_gp = ctx.enter_context(nc.semaphore(name="s_gp"))
        s_ck = ctx.enter_context(nc.semaphore(name="s_ck"))
        s_q8 = ctx.enter_context(nc.semaphore(name="s_q8"))
        s_pe = ctx.enter_context(nc.semaphore(name="s_pe"))
        s_dve = ctx.enter_context(nc.semaphore(name="s_dve"))
        s_fin = ctx.enter_context(nc.semaphore(name="s_fin"))

        block = ctx.enter_context(nc.Block())

        wq4 = wqs.ap().rearrange("p (t i f) -> p t i f", t=SPC, i=2)
        bs4 = bss.ap().rearrange("p (i r f) -> p i r f", i=2, r=NGRP)
        gp3 = gpt.ap().rearrange("p (g f) -> p g f", g=NGRP)

        def wchunk(k):
            return slice(k * 2 * SPAD * 2, (k + 1) * 2 * SPAD * 2)

        @block.sync
        def _(sync):
            sync.dma_start(bss.ap(), bs).then_inc(s_bs, 16)
            for k in (0, 2, 4, 6):
                sync.dma_start(wqs.ap()[:, wchunk(k)], wq[:, wchunk(k)]).then_inc(s_w[k], 16)
            sync.wait_ge(s_dve, 2 + NGRP)
            sync.dma_start(acc_out, acc.ap()).then_inc(s_fin, 16)

        @block.scalar
        def _(scalar):
            scalar.dma_start(wqs.ap()[:, wchunk(1)], wq[:, wchunk(1)]).then_inc(s_w[1], 16)
            scalar.dma_start(gpt.ap(), gp).then_inc(s_gp, 16)
            for k in (3, 5, 7):
                scalar.dma_start(wqs.ap()[:, wchunk(k)], wq[:, wchunk(k)]).then_inc(s_w[k], 16)

        @block.gpsimd
        def _(gpsimd):
            gpsimd.dma_start(ckt.ap(), ck).then_inc(s_ck, 16)
            gpsimd.dma_start(q8t.ap(), q8).then_inc(s_q8, 16)

        @block.tensor
        def _(tensor):
            tensor.wait_ge(s_dve, 1)
            for _ in range(3):
                tensor.matmul(psw.ap(), wu.ap()[:, 0:128], wu.ap(),
                              start=True, stop=True)
            tensor.wait_ge(s_bs, 16)
            for t in range(SPC):
                g, r = t // 4, t % 4
                if t % 2 == 0:
                    tensor.wait_ge(s_w[t // 2], 16)
                mm = tensor.matmul(
                    ps[g].ap()[0:8, 0:SCOLS],
                    bs4[:, :, r, 0:8],
                    wq4[:, t, :, 0:SCOLS],
                    start=(r == 0), stop=(r == 3),
                    perf_mode=mybir.MatmulPerfMode.DoubleRow,
                )
                if r == 3:
                    mm.then_inc(s_pe, 1)
            tensor.wait_ge(s_ck, 16)
            tensor.matmul(psq.ap(), ckt.ap(), ckt.ap(),
                          start=True, stop=True).then_inc(s_pe, 1)

        @block.vector
        def _(vector):
            vector.memset(wu.ap(), 0.125).then_inc(s_dve, 1)
            for g in range(NGRP):
                if g == 0:
                    vector.wait_ge(s_gp, 16)
                vector.wait_ge(s_pe, g + 1)
                vector.scalar_tensor_tensor(
                    out=scrap.ap()[0:8, 0:SCOLS],
                    in0=ps[g].ap()[0:8, 0:SCOLS],
                    scalar=1.0,
                    in1=gp3[:, g, 0:SCOLS],
                    op0=AOT.mult,
                    op1=AOT.mult,
                    accum_out=acc.ap()[:, g:g + 1],
                ).then_inc(s_dve, 1)
            vector.wait_ge(s_pe, NGRP + 1)
            vector.wait_ge(s_q8, 16)
            vector.scalar_tensor_tensor(
                out=scrap.ap()[0:8, 0:8],
                in0=psq.ap(),
                scalar=1.0,
                in1=q8t.ap(),
                op0=AOT.mult,
                op1=AOT.mult,
                accum_out=acc.ap()[:, 4:5],
            ).then_inc(s_dve, 1)

    return nc


def _precompute(coeff, cost_mat, ts, w, num_steps):
    """Host-side prep: fp8 w tiles, fp8 basis variants, bf16 linearized Gp,
    quad operands."""
    N = int(num_steps)
    ts = np.asarray(ts, np.float32)
    coeff = np.asarray(coeff, np.float32)
    w = np.asarray(w, np.float32)

    times = np.linspace(np.float32(ts[0]), np.float32(ts[-1]), N, dtype=np.float32)
    k = np.searchsorted(ts[1:-1], times, side="left")
    counts = np.bincount(k, minlength=NUM_SEG)
    starts = np.concatenate([[0], np.cumsum(counts)[:-1]]).astype(np.int64)
    assert counts.max() <= UB * QB

    # G[seg, s, e]: per-output-row polynomial coefficients in dt^e
    d_of_s = np.array([0, 0, 0, 1, 1, 1, 2, 2, 2, 3, 3, 3, 0, 1])
    a_of_s = np.array([0, 1, 2, 0, 1, 2, 0, 1, 2, 0, 1, 2, 3, 3])
    G = np.zeros((NUM_SEG, 14, NC8), np.float64)
    for s in range(14):
        d, a = int(d_of_s[s]), int(a_of_s[s])
        for e in range(NC8 - d):
            G[:, s, e] = _falling(e + d, d) * coeff[a, :, e + d].astype(np.float64)

    h = (np.float64(ts[-1]) - np.float64(ts[0])) / (N - 1)
    ts64 = ts.astype(np.float64)

    # per-u-block midpoint linearization: ref ~= Gp0 + (q/QB)*Gp1
    u = np.arange(UB)
    idx = np.minimum(starts[:, None] + QB * u[None, :], N - 1)   # (128, 31)
    dtb = times[idx].astype(np.float64) - ts64[:NUM_SEG, None]
    m = dtb + (QB // 2) * h                                       # midpoints
    e = np.arange(NC8)
    mpow = m[:, :, None] ** e[None, None, :]                      # (128, 31, 8)
    dpow = np.zeros_like(mpow)
    dpow[:, :, 1:] = e[1:][None, None, :] * (m[:, :, None] ** (e[1:] - 1)[None, None, :])
    refm = np.einsum("kse,kue->ksu", G, mpow)                     # (128, 14, 31)
    refpm = np.einsum("kse,kue->ksu", G, dpow)
    gp1 = QB * h * refpm
    gp0 = refm - (QB // 2) * h * refpm

    bf = mybir.dt.np(BF16)
    f8np = mybir.dt.np(F8)

    # basis variants: bs[k, i*32 + r*8 + c]; c==2r -> 1, c==2r+1 -> q/QB
    bs_host = np.zeros((128, 64), np.float32)
    kk = np.arange(128, dtype=np.float32)
    for i in range(2):
        for r in range(NGRP):
            bs_host[:, i * 32 + r * 8 + 2 * r] = 1.0
            bs_host[:, i * 32 + r * 8 + 2 * r + 1] = (i * 128.0 + kk) / QB
    bs_host = bs_host.astype(f8np)

    w_scaled = (w[14:].astype(np.float32) * np.float32(W_SCALE)).astype(f8np)

    cost_mat = np.asarray(cost_mat, np.float32)
    q8b = np.ascontiguousarray(cost_mat[:NC8, :NC8])

    in_maps = []
    for c in range(N_CORES):
        sl = slice(c * SPC, (c + 1) * SPC)
        wq_core = np.zeros((128, SPC, 2, SPAD), f8np)
        for t in range(SPC):
            g = c * SPC + t
            st, cnt = int(starts[g]), int(counts[g])
            blk = np.zeros((UB * QB * 14,), f8np)
            blk[: 14 * cnt] = w_scaled[14 * st: 14 * (st + cnt)]
            # step = u*256 + i*128 + k ; flat = 14*step + s
            blk = blk.reshape(UB, 2, 128, 14).transpose(2, 1, 0, 3)  # (k, i, u, s)
            wq_core[:, t, :, 0:SCOLS] = blk.reshape(128, 2, SCOLS)

        # gp layout: [2r+q, g*SPAD + u*14+s] for seg = 16c + 4g + r
        gp_host = np.zeros((8, NGRP, SPAD), np.float64)
        for t in range(SPC):
            g, r = t // 4, t % 4
            seg = c * SPC + t
            gp_host[2 * r + 0, g, 0:SCOLS] = gp0[seg].T.reshape(SCOLS)
            gp_host[2 * r + 1, g, 0:SCOLS] = gp1[seg].T.reshape(SCOLS)

        in_maps.append({
            "wq": np.ascontiguousarray(wq_core.reshape(128, WFREE)),
            "bs": bs_host,
            "gp": np.ascontiguousarray(gp_host.reshape(8, NGRP * SPAD)).astype(bf),
            "ck": np.ascontiguousarray(
                coeff[:4, sl, :].reshape(4 * SPC, NC8)).astype(np.float32),
            "q8": q8b,
        })
    return in_maps


def _install_ntff_hook_shim():
    """The agent image lacks ``antenv.axon_hooks``; recreate it so
    run_bass_kernel_spmd's trace=True path can find the NTFF profile hook
    (test-only; the grading path never passes _trace)."""
    import sys, types
    if "antenv.axon_hooks" in sys.modules:
        return
    import antenv
    mod = types.ModuleType("antenv.axon_hooks")
    _h = [None]
    mod.set_axon_ntff_profile_hook = lambda h: _h.__setitem__(0, h)
    mod.get_axon_ntff_profile_hook = lambda: _h[0]
    sys.modules["antenv.axon_hooks"] = mod
    antenv.axon_hooks = mod
    try:
        from trn_agent_boot.trn_boot import _ntff_profile_via_ctypes
        mod.set_axon_ntff_profile_hook(
            _ntff_profile_via_ctypes("/opt/axon/libaxon_pjrt.so"))
    except Exception as e:
        print("ntff hook shim failed:", e)


def kernel(coeff, cost_mat, ts, x0, w_reg, rho, p, num_steps,
           _trace=False, _trace_cores=None):
    global LAST_RESULTS
    coeff = np.asarray(coeff)
    cost_mat = np.asarray(cost_mat)
    ts = np.asarray(ts)
    x0 = np.asarray(x0)
    w_reg = np.asarray(w_reg)
    assert int(p) == 4 and int(num_steps) == 1_000_000

    cost_mat32 = np.asarray(cost_mat, np.float32)
    q8b = cost_mat32[:NC8, :NC8]
    kron_ok = np.array_equal(
        cost_mat32, np.kron(np.eye(NUM_SEG, dtype=np.float32), q8b))
    in_maps = _precompute(coeff, cost_mat, ts, w_reg, num_steps)
    nc = _build_nc()
    kwargs = {}
    if _trace:
        _install_ntff_hook_shim()
        kwargs = dict(trace=True, trace_cores=_trace_cores or [0])
    res = run_bass_kernel_spmd(nc, in_maps, list(range(N_CORES)), **kwargs)
    LAST_RESULTS = res

    quad = 0.0
    reg = 0.0
    for c in range(N_CORES):
        acc = np.asarray(res.results[c]["acc_out"], np.float64)
        reg += acc[:, :NGRP].sum() / W_SCALE
        quad += acc[:, 4].sum()
    reg += float(np.asarray(w_reg[:14], np.float64) @ np.asarray(x0, np.float64))
    if not kron_ok:
        # cost_mat without the expected kron structure: the on-device quad
        # fast path does not apply; recompute the (tiny) quadratic exactly.
        C = np.asarray(coeff, np.float64)[:4].reshape(4, -1)
        quad = float(np.einsum("pi,ij,pj->", C, np.asarray(cost_mat, np.float64), C))
    return np.float32(quad + float(rho) * reg)


# revision 11
# speedup vs baseline: 1.7038x; 1.0670x over previous
"""Trainium2 Bass kernel for nn_MinJerkReg (min-jerk quadratic cost + trajectory
regularizer loss).

Math
----
reference() = quad + rho * reg where
  quad = sum_{p,i,j} C[p,i] cost_mat[i,j] C[p,j],   C = coeff[:4] reshaped (4,1024)
  reg  = w_reg[:14] @ x0 + sum_{n,s} w_reg[14+14n+s] * ref[s,n]
  ref[s,n] = degree-<=7 polynomial of the segment-local time dt_n.

Device decomposition (8 cores, 16 of the 128 segments each, ~125k steps/core):
  Steps within a segment are blocked (u, q) with q in [0,256).  Around each
  block midpoint the polynomial is linearized: ref ~= Gp0[u,s] + (q/256)*
  Gp1[u,s] (the quadratic remainder is ~1e-4 relative -- far below the fp8
  noise of the w stream).  One DoubleRow fp8 matmul per segment contracts
  q (K=256) between a stationary basis {1, q/256} and the moving w tile
  [128, 2, 434], so the heavy w multiply-reduce runs on the tensor engine
  at 2 elem/cell/cycle.  Four consecutive segments share one PSUM bank:
  segment r of a group carries its basis in lhsT column pair (2r, 2r+1)
  with zeros elsewhere, so PSUM accumulation merges the four matmuls into
  disjoint rows of one [8, 434] tile (walrus rejects DoubleRow +
  tile_position, so col-group spreading is done through the weights).  A
  single fused DVE scalar_tensor_tensor per group multiplies by the
  host-precomputed bf16 Gp tile and reduces into acc[0:8, g]; the host sums
  those.  quad: one tiny f32r matmul + fused DVE reduce against Q8 (host
  falls back to an exact f64 einsum if cost_mat loses its kron structure).
  w is quantized host-side to fp8e4 (x256; random-sign noise into a
  14M-term dot, ~1e-5 relative effect) and streamed in 8 chunks of 229KB
  alternating across the two HWDGE rings (sync/scalar) so descriptor
  emission never starves the SDMA engines; small operands ride the same
  rings (bs/gp) or SWDGE (ck/q8).  Three bf16 warmup matmuls at t=0 keep
  the PE HAM clock-gate warming while the first w chunk lands.

This toolchain permits exactly ONE semaphore wait per instruction, so extra
dependencies are standalone wait_ge instructions (raw Bass, no Tile).
"""

import numpy as np

import concourse.bass as bass
import concourse.mybir as mybir
from concourse.bass_utils import run_bass_kernel_spmd

F32 = mybir.dt.float32
F8 = mybir.dt.float8e4
BF16 = mybir.dt.bfloat16
F32R = mybir.dt.float32r
W_SCALE = 256.0
AOT = mybir.AluOpType

N_CORES = 8
NUM_SEG = 128
SPC = NUM_SEG // N_CORES              # 16 segments per core
ORDER = 7
NC8 = ORDER + 1
QB = 256                               # q (contraction) steps per u-block
UB = 31                                # u-blocks per segment (31*256 >= 7813)
SCOLS = UB * 14                        # 434 real rhs columns per segment
SPAD = 448                             # padded to a 16-elem multiple
NCHUNK = 8                             # w DMA chunks (2 segments each)
NGRP = 4                               # PSUM groups (4 segments each)
WFREE = SPC * 2 * SPAD                 # 14336 fp8 bytes per partition

# module global: last BassKernelResults (for test harness introspection)
LAST_RESULTS = None


def _falling(j, d):
    return float(np.prod(np.arange(j, j - d, -1))) if j >= d else 0.0


def _build_nc():
    nc = bass.Bass(trn_type="TRN2", num_devices=N_CORES, debug=False)
    # wq cols 0:64 carry the fp8 basis variants (ride chunk 0); w data after.
    wq = nc.dram_tensor("wq", [128, 64 + WFREE], F8, kind="ExternalInput").ap()
    gp = nc.dram_tensor("gp", [8, NGRP * SPAD], BF16, kind="ExternalInput").ap()
    ck = nc.dram_tensor("ck", [64, 8], F32R, kind="ExternalInput").ap()
    q8 = nc.dram_tensor("q8", [8, 8], F32, kind="ExternalInput").ap()
    acc_out = nc.dram_tensor("acc_out", [8, 5], F32, kind="ExternalOutput").ap()

    import contextlib
    ctx = contextlib.ExitStack()
    with ctx:
        wqs = ctx.enter_context(nc.sbuf_tensor([128, 64 + WFREE], F8))
        gpt = ctx.enter_context(nc.sbuf_tensor([8, NGRP * SPAD], BF16))
        wu = ctx.enter_context(nc.sbuf_tensor([128, 512], BF16))
        scrap = ctx.enter_context(nc.sbuf_tensor([128, 5 * 512], F32))
        ckt = ctx.enter_context(nc.sbuf_tensor([64, 8], F32R))
        q8t = ctx.enter_context(nc.sbuf_tensor([8, 8], F32))
        acc = ctx.enter_context(nc.sbuf_tensor([8, 5], F32))
        ps = [ctx.enter_context(nc.psum_tensor(f"ps{g}", [128, 512], F32))
              for g in range(NGRP)]
        psw = ctx.enter_context(nc.psum_tensor("psw", [128, 512], F32))
        psq = ctx.enter_context(nc.psum_tensor("psq", [8, 8], F32))

        # per-DMA completion sems (engine-level inc interleaving across
        # consecutive DMAs on one ring makes shared counters unsound),
        # plus PE / DVE progress counters
        s_w = [ctx.enter_context(nc.semaphore(name=f"s_w{k}")) for k in range(NCHUNK)]
        s_gp = ctx.enter_context(nc.semaphore(name="s_gp"))
        s_ck = ctx.enter_context(nc.semaphore(name="s_ck"))
        s_q8 = ctx.enter_context(nc.semaphore(name="s_q8"))
        s_pe = ctx.enter_context(nc.semaphore(name="s_pe"))
        s_dve = ctx.enter_context(nc.semaphore(name="s_dve"))
        s_fin = ctx.enter_context(nc.semaphore(name="s_fin"))

        block = ctx.enter_context(nc.Block())

        wq4 = wqs.ap()[:, 64:].rearrange("p (t i f) -> p t i f", t=SPC, i=2)
        bs4 = wqs.ap()[:, 0:64].rearrange("p (i r f) -> p i r f", i=2, r=NGRP)
        gp3 = gpt.ap().rearrange("p (g f) -> p g f", g=NGRP)

        def wchunk(k):
            lo = 0 if k == 0 else 64 + k * 2 * SPAD * 2
            return slice(lo, 64 + (k + 1) * 2 * SPAD * 2)

        # sync ring: c0(+basis), gp, c2, c4, c6, ck, acc_out
        @block.sync
        def _(sync):
            sync.dma_start(wqs.ap()[:, wchunk(0)], wq[:, wchunk(0)]).then_inc(s_w[0], 16)
            sync.dma_start(gpt.ap(), gp).then_inc(s_gp, 16)
            for k in (2, 4, 6):
                sync.dma_start(wqs.ap()[:, wchunk(k)], wq[:, wchunk(k)]).then_inc(s_w[k], 16)
            sync.dma_start(ckt.ap(), ck).then_inc(s_ck, 16)
            sync.wait_ge(s_dve, 6)
            sync.dma_start(acc_out, acc.ap()).then_inc(s_fin, 16)

        # scalar ring: c1, c3, c5, c7, q8
        @block.scalar
        def _(scalar):
            for k in (1, 3, 5, 7):
                scalar.dma_start(wqs.ap()[:, wchunk(k)], wq[:, wchunk(k)]).then_inc(s_w[k], 16)
            scalar.dma_start(q8t.ap(), q8).then_inc(s_q8, 16)

        @block.tensor
        def _(tensor):
            tensor.wait_ge(s_dve, 1)
            for _ in range(4):
                tensor.matmul(psw.ap(), wu.ap()[:, 0:128], wu.ap(),
                              start=True, stop=True)
            for t in range(SPC):
                g, r = t // 4, t % 4
                if r == 0:
                    tensor.wait_ge(s_w[2 * g], 16)
                    tensor.wait_ge(s_w[2 * g + 1], 16)
                mm = tensor.matmul(
                    ps[g].ap()[0:8, 0:SCOLS],
                    bs4[:, :, r, 0:8],
                    wq4[:, t, :, 0:SCOLS],
                    start=(r == 0), stop=(r == 3),
                    perf_mode=mybir.MatmulPerfMode.DoubleRow,
                )
                if r == 3:
                    mm.then_inc(s_pe, 1)
            tensor.wait_ge(s_ck, 16)
            tensor.matmul(psq.ap(), ckt.ap(), ckt.ap(),
                          start=True, stop=True).then_inc(s_pe, 1)

        @block.vector
        def _(vector):
            vector.memset(wu.ap(), 0.125).then_inc(s_dve, 1)
            for g in range(NGRP):
                if g == 0:
                    vector.wait_ge(s_gp, 16)
                vector.wait_ge(s_pe, g + 1)
                vector.scalar_tensor_tensor(
                    out=scrap.ap()[0:8, g * 512:g * 512 + SCOLS],
                    in0=ps[g].ap()[0:8, 0:SCOLS],
                    scalar=1.0,
                    in1=gp3[:, g, 0:SCOLS],
                    op0=AOT.mult,
                    op1=AOT.mult,
                    accum_out=acc.ap()[:, g:g + 1],
                ).then_inc(s_dve, 1)
            vector.wait_ge(s_pe, NGRP + 1)
            vector.wait_ge(s_q8, 16)
            vector.scalar_tensor_tensor(
                out=scrap.ap()[0:8, 2048:2056],
                in0=psq.ap(),
                scalar=1.0,
                in1=q8t.ap(),
                op0=AOT.mult,
                op1=AOT.mult,
                accum_out=acc.ap()[:, 4:5],
            ).then_inc(s_dve, 1)

    return nc


def _precompute(coeff, cost_mat, ts, w, num_steps):
    """Host-side prep: fp8 w tiles, fp8 basis variants, bf16 linearized Gp,
    quad operands."""
    N = int(num_steps)
    ts = np.asarray(ts, np.float32)
    coeff = np.asarray(coeff, np.float32)
    w = np.asarray(w, np.float32)

    times = np.linspace(np.float32(ts[0]), np.float32(ts[-1]), N, dtype=np.float32)
    k = np.searchsorted(ts[1:-1], times, side="left")
    counts = np.bincount(k, minlength=NUM_SEG)
    starts = np.concatenate([[0], np.cumsum(counts)[:-1]]).astype(np.int64)
    assert counts.max() <= UB * QB

    # G[seg, s, e]: per-output-row polynomial coefficients in dt^e
    d_of_s = np.array([0, 0, 0, 1, 1, 1, 2, 2, 2, 3, 3, 3, 0, 1])
    a_of_s = np.array([0, 1, 2, 0, 1, 2, 0, 1, 2, 0, 1, 2, 3, 3])
    G = np.zeros((NUM_SEG, 14, NC8), np.float64)
    for s in range(14):
        d, a = int(d_of_s[s]), int(a_of_s[s])
        for e in range(NC8 - d):
            G[:, s, e] = _falling(e + d, d) * coeff[a, :, e + d].astype(np.float64)

    h = (np.float64(ts[-1]) - np.float64(ts[0])) / (N - 1)
    ts64 = ts.astype(np.float64)

    # per-u-block midpoint linearization: ref ~= Gp0 + (q/QB)*Gp1
    u = np.arange(UB)
    idx = np.minimum(starts[:, None] + QB * u[None, :], N - 1)   # (128, 31)
    dtb = times[idx].astype(np.float64) - ts64[:NUM_SEG, None]
    m = dtb + (QB // 2) * h                                       # midpoints
    e = np.arange(NC8)
    mpow = m[:, :, None] ** e[None, None, :]                      # (128, 31, 8)
    dpow = np.zeros_like(mpow)
    dpow[:, :, 1:] = e[1:][None, None, :] * (m[:, :, None] ** (e[1:] - 1)[None, None, :])
    refm = np.einsum("kse,kue->ksu", G, mpow)                     # (128, 14, 31)
    refpm = np.einsum("kse,kue->ksu", G, dpow)
    gp1 = QB * h * refpm
    gp0 = refm - (QB // 2) * h * refpm

    bf = mybir.dt.np(BF16)
    f8np = mybir.dt.np(F8)

    # basis variants: bs[k, i*32 + r*8 + c]; c==2r -> 1, c==2r+1 -> q/QB
    bs_host = np.zeros((128, 64), np.float32)
    kk = np.arange(128, dtype=np.float32)
    for i in range(2):
        for r in range(NGRP):
            bs_host[:, i * 32 + r * 8 + 2 * r] = 1.0
            bs_host[:, i * 32 + r * 8 + 2 * r + 1] = (i * 128.0 + kk) / QB
    bs_host = bs_host.astype(f8np)

    w_scaled = (w[14:].astype(np.float32) * np.float32(W_SCALE)).astype(f8np)

    cost_mat = np.asarray(cost_mat, np.float32)
    q8b = np.ascontiguousarray(cost_mat[:NC8, :NC8])

    in_maps = []
    for c in range(N_CORES):
        sl = slice(c * SPC, (c + 1) * SPC)
        wq_core = np.zeros((128, 64 + WFREE), f8np)
        wq_core[:, 0:64] = bs_host
        wv = wq_core[:, 64:].reshape(128, SPC, 2, SPAD)
        for t in range(SPC):
            g = c * SPC + t
            st, cnt = int(starts[g]), int(counts[g])
            blk = np.zeros((UB * QB * 14,), f8np)
            blk[: 14 * cnt] = w_scaled[14 * st: 14 * (st + cnt)]
            # step = u*256 + i*128 + k ; flat = 14*step + s
            blk = blk.reshape(UB, 2, 128, 14).transpose(2, 1, 0, 3)  # (k, i, u, s)
            wv[:, t, :, 0:SCOLS] = blk.reshape(128, 2, SCOLS)

        # gp layout: [2r+q, g*SPAD + u*14+s] for seg = 16c + 4g + r
        gp_host = np.zeros((8, NGRP, SPAD), np.float64)
        for t in range(SPC):
            g, r = t // 4, t % 4
            seg = c * SPC + t
            gp_host[2 * r + 0, g, 0:SCOLS] = gp0[seg].T.reshape(SCOLS)
            gp_host[2 * r + 1, g, 0:SCOLS] = gp1[seg].T.reshape(SCOLS)

        in_maps.append({
            "wq": wq_core,
            "gp": np.ascontiguousarray(gp_host.reshape(8, NGRP * SPAD)).astype(bf),
            "ck": np.ascontiguousarray(
                coeff[:4, sl, :].reshape(4 * SPC, NC8)).astype(np.float32),
            "q8": q8b,
        })
    return in_maps


def _install_ntff_hook_shim():
    """The agent image lacks ``antenv.axon_hooks``; recreate it so
    run_bass_kernel_spmd's trace=True path can find the NTFF profile hook
    (test-only; the grading path never passes _trace)."""
    import sys, types
    if "antenv.axon_hooks" in sys.modules:
        return
    import antenv
    mod = types.ModuleType("antenv.axon_hooks")
    _h = [None]
    mod.set_axon_ntff_profile_hook = lambda h: _h.__setitem__(0, h)
    mod.get_axon_ntff_profile_hook = lambda: _h[0]
    sys.modules["antenv.axon_hooks"] = mod
    antenv.axon_hooks = mod
    try:
        from trn_agent_boot.trn_boot import _ntff_profile_via_ctypes
        mod.set_axon_ntff_profile_hook(
            _ntff_profile_via_ctypes("/opt/axon/libaxon_pjrt.so"))
    except Exception as e:
        print("ntff hook shim failed:", e)


def kernel(coeff, cost_mat, ts, x0, w_reg, rho, p, num_steps,
           _trace=False, _trace_cores=None):
    global LAST_RESULTS
    coeff = np.asarray(coeff)
    cost_mat = np.asarray(cost_mat)
    ts = np.asarray(ts)
    x0 = np.asarray(x0)
    w_reg = np.asarray(w_reg)
    assert int(p) == 4 and int(num_steps) == 1_000_000

    cost_mat32 = np.asarray(cost_mat, np.float32)
    q8b = cost_mat32[:NC8, :NC8]
    kron_ok = np.array_equal(
        cost_mat32, np.kron(np.eye(NUM_SEG, dtype=np.float32), q8b))
    in_maps = _precompute(coeff, cost_mat, ts, w_reg, num_steps)
    nc = _build_nc()
    kwargs = {}
    if _trace:
        _install_ntff_hook_shim()
        kwargs = dict(trace=True, trace_cores=_trace_cores or [0])
    res = run_bass_kernel_spmd(nc, in_maps, list(range(N_CORES)), **kwargs)
    LAST_RESULTS = res

    quad = 0.0
    reg = 0.0
    for c in range(N_CORES):
        acc = np.asarray(res.results[c]["acc_out"], np.float64)
        reg += acc[:, :NGRP].sum() / W_SCALE
        quad += acc[:, 4].sum()
    reg += float(np.asarray(w_reg[:14], np.float64) @ np.asarray(x0, np.float64))
    if not kron_ok:
        # cost_mat without the expected kron structure: the on-device quad
        # fast path does not apply; recompute the (tiny) quadratic exactly.
        C = np.asarray(coeff, np.float64)[:4].reshape(4, -1)
        quad = float(np.einsum("pi,ij,pj->", C, np.asarray(cost_mat, np.float64), C))
    return np.float32(quad + float(rho) * reg)
